# revision 23
# baseline (speedup 1.0000x reference)
"""Trainium2 Bass kernel for nn_EnhancedMemoryManager (scatter_memory).

Contract: kernel(**inputs) takes FULL unsharded inputs and returns the FULL
outputs (new_user_prototypes, new_shared, new_ic), matching reference().

Sharding strategy (hardcoded for shapes:
  user_prototypes [50000,16,128], features [8192,50,128], success_mask
  [8192,50], interaction_count [50000], shared_prototypes [1,16,128],
  user_idx [8192]):
 - batch rows are split contiguously: core d gets rows [d*1024,(d+1)*1024);
   the prototype/ic rows for those users are routed (host-side gather) to
   that core (expert/data-parallel routing per user ownership).
 - the 41808 untouched user rows are split 5226/core and passed through the
   device (DMA copy) so the full output streams out of the 8 cores.
 - the shared_prototypes update (mean over the full batch) is AllReduced
   across the 8 cores; every core computes the final new_shared.
"""

from contextlib import ExitStack

import numpy as np

import concourse.bacc as bacc
import concourse.bass as bass
import concourse.mybir as mybir
import concourse.tile as tile
from concourse import bass_utils

N_CORES = 8
NUM_USERS, P, D = 50000, 16, 128
B, S = 8192, 50
BPC = B // N_CORES              # 1024 batch rows per core
NCHUNK = BPC // 128             # 8 chunks of 128 rows
PASS_TOTAL = NUM_USERS - B      # 41808 untouched rows
PASS_ROWS = PASS_TOTAL // N_CORES  # 5226 per core
MOM = 0.9
EPS_MASK = 1e-6
EPS_NORM = 1e-12
F32 = mybir.dt.float32


def build_module(pass_through: bool = True, use_collective: bool = True,
                 mul_engine: str = "vector", gp_mul_chunks: int = 0,
                 gp_blend_chunks: int = 0):
    nc = bacc.Bacc("TRN2", target_bir_lowering=False, debug=False,
                   num_devices=N_CORES)

    feat_d = nc.dram_tensor("features", [BPC, S * D], F32, kind="ExternalInput")
    mask_d = nc.dram_tensor("mask", [BPC, S], F32, kind="ExternalInput")
    cur_d = nc.dram_tensor("cur", [BPC, P * D], F32, kind="ExternalInput")
    ic_d = nc.dram_tensor("ic", [BPC, 1], F32, kind="ExternalInput")
    shared_d = nc.dram_tensor("shared", [P, D], F32, kind="ExternalInput")
    nb_d = nc.dram_tensor("new_batch", [BPC, P * D], F32, kind="ExternalOutput")
    nic_d = nc.dram_tensor("new_ic", [BPC, 1], F32, kind="ExternalOutput")
    nsh_d = nc.dram_tensor("new_shared", [P, D], F32, kind="ExternalOutput")
    if pass_through:
        pin_d = nc.dram_tensor("pass_in", [PASS_ROWS, P * D], F32,
                               kind="ExternalInput")
        pic_d = nc.dram_tensor("pass_ic_in", [PASS_ROWS, 1], F32,
                               kind="ExternalInput")
        pout_d = nc.dram_tensor("pass_out", [PASS_ROWS, P * D], F32,
                                kind="ExternalOutput")
        picout_d = nc.dram_tensor("pass_ic_out", [PASS_ROWS, 1], F32,
                                  kind="ExternalOutput")
    if not use_collective:
        partial_d = nc.dram_tensor("partial", [1, D], F32, kind="ExternalOutput")

    mul_eng = {"vector": nc.vector, "gpsimd": nc.gpsimd}[mul_engine]

    with tile.TileContext(nc) as tc, ExitStack() as ctx:
        const = ctx.enter_context(tc.tile_pool(name="const", bufs=1))
        featp = ctx.enter_context(tc.tile_pool(name="featp", bufs=3))
        curp = ctx.enter_context(tc.tile_pool(name="curp", bufs=3))
        outp = ctx.enter_context(tc.tile_pool(name="outp", bufs=3))
        small = ctx.enter_context(tc.tile_pool(name="small", bufs=3))
        psump = ctx.enter_context(tc.tile_pool(name="psump", bufs=1, space="PSUM"))
        dramp = ctx.enter_context(tc.tile_pool(name="dramp", bufs=1, space="DRAM"))

        # whole-shard loads of the small tensors
        mask_sb = const.tile([128, NCHUNK * S], F32)
        nc.sync.dma_start(
            mask_sb[:].rearrange("p (k s) -> p k s", k=NCHUNK),
            mask_d[:, :].rearrange("(k p) s -> p k s", p=128),
        )
        ic_sb = const.tile([128, NCHUNK], F32)
        nc.sync.dma_start(
            ic_sb[:], ic_d[:, :].rearrange("(k p) o -> p (k o)", p=128))
        shared_sb = const.tile([P, D], F32)
        nc.sync.dma_start(shared_sb[:], shared_d[:, :])
        ones_sb = const.tile([128, 1], F32)
        nc.vector.memset(ones_sb[:], 1.0)
        ones16 = const.tile([1, P], F32)
        nc.vector.memset(ones16[:], 1.0)

        psum_part = psump.tile([1, D], F32)

        # batched per-row scalars for ALL chunks at once: [128, NCHUNK]
        m_all = const.tile([128, NCHUNK], F32)
        nc.vector.tensor_scalar(m_all[:], ic_sb[:], 0.001, MOM,
                                mybir.AluOpType.mult, mybir.AluOpType.add)
        nc.vector.tensor_scalar(m_all[:], m_all[:], MOM, 0.99,
                                mybir.AluOpType.max, mybir.AluOpType.min)
        om_all = const.tile([128, NCHUNK], F32)
        nc.vector.tensor_scalar(om_all[:], m_all[:], -1.0, 1.0,
                                mybir.AluOpType.mult, mybir.AluOpType.add)
        nic_all = const.tile([128, NCHUNK], F32)
        nc.vector.tensor_scalar_add(nic_all[:], ic_sb[:], 1.0)
        nc.sync.dma_start(
            nic_d[:, :].rearrange("(k p) o -> p (k o)", p=128), nic_all[:])
        # msum/inv for all chunks: [128, NCHUNK]
        invm_all = const.tile([128, NCHUNK], F32)
        nc.vector.tensor_reduce(
            invm_all[:], mask_sb[:].rearrange("p (k s) -> p k s", k=NCHUNK),
            axis=mybir.AxisListType.X, op=mybir.AluOpType.add)
        nc.vector.tensor_scalar_max(invm_all[:], invm_all[:], EPS_MASK)
        nc.vector.reciprocal(invm_all[:], invm_all[:])

        for k in range(NCHUNK):
            feat_t = featp.tile([128, S * D], F32)
            nc.sync.dma_start(feat_t[:], feat_d[bass.ts(k, 128), :])
            cur_t = curp.tile([128, P * D], F32)
            nc.sync.dma_start(cur_t[:], cur_d[bass.ts(k, 128), :])

            mask_k = mask_sb[:, bass.ts(k, S)]                      # [128, S]
            m_t = m_all[:, k:k + 1]
            om_t = om_all[:, k:k + 1]

            # masked sum over history: prod = feat * mask (broadcast over d)
            feat3 = feat_t[:].rearrange("p (s d) -> p s d", s=S)
            mask_b = mask_k.unsqueeze(2).broadcast_to([128, S, D])
            if gp_mul_chunks == -1:  # split each chunk's mul DVE/gpsimd
                h = S // 2
                nc.vector.tensor_tensor(
                    feat3[:, :h], feat3[:, :h], mask_b[:, :h],
                    mybir.AluOpType.mult)
                nc.gpsimd.tensor_tensor(
                    feat3[:, h:], feat3[:, h:], mask_b[:, h:],
                    mybir.AluOpType.mult)
            else:
                meng = nc.gpsimd if k < gp_mul_chunks else mul_eng
                meng.tensor_tensor(feat3, feat3, mask_b, mybir.AluOpType.mult)

            # usum[p, d] = sum_s prod[p, s, d]
            usum = small.tile([128, D], F32)
            nc.vector.tensor_reduce(
                usum[:], feat_t[:].rearrange("p (s d) -> p d s", s=S),
                axis=mybir.AxisListType.X, op=mybir.AluOpType.add)

            # upd = usum / max(msum, eps)
            upd = small.tile([128, D], F32)
            nc.vector.tensor_scalar_mul(upd[:], usum[:], invm_all[:, k:k + 1])

            # l2 normalize upd along d
            sq = small.tile([128, D], F32)
            nrm2 = small.tile([128, 1], F32)
            nc.scalar.activation(sq[:], upd[:],
                                 mybir.ActivationFunctionType.Square,
                                 accum_out=nrm2[:])
            nrm = small.tile([128, 1], F32)
            nc.scalar.sqrt(nrm[:], nrm2[:])
            nc.vector.tensor_scalar_max(nrm[:], nrm[:], EPS_NORM)
            invn = small.tile([128, 1], F32)
            nc.vector.reciprocal(invn[:], nrm[:])
            updn = small.tile([128, D], F32)
            nc.vector.tensor_scalar_mul(updn[:], upd[:], invn[:])

            # partial sum over batch rows of updn (for shared update): PE
            nc.tensor.matmul(psum_part[:], lhsT=ones_sb[:], rhs=updn[:],
                             start=(k == 0), stop=(k == NCHUNK - 1))

            # v = updn * (1 - m); new = cur * m + v
            v_t = small.tile([128, D], F32)
            nc.vector.tensor_scalar_mul(v_t[:], updn[:], om_t)
            out_t = outp.tile([128, P * D], F32)
            beng = nc.gpsimd if k < gp_blend_chunks else nc.vector
            beng.scalar_tensor_tensor(
                out_t[:].rearrange("p (q d) -> p q d", q=P),
                cur_t[:].rearrange("p (q d) -> p q d", q=P),
                m_t,
                v_t[:].unsqueeze(1).broadcast_to([128, P, D]),
                mybir.AluOpType.mult, mybir.AluOpType.add)
            nc.sync.dma_start(nb_d[bass.ts(k, 128), :], out_t[:])

        # ---- shared prototype update ----
        part_sb = small.tile([1, D], F32)
        nc.vector.tensor_copy(part_sb[:], psum_part[:])

        if use_collective:
            cc_in = dramp.tile([1, D], F32)
            cc_out = dramp.tile([1, D], F32, addr_space="Shared")
            nc.sync.dma_start(cc_in[:], part_sb[:])
            nc.gpsimd.collective_compute(
                "AllReduce", mybir.AluOpType.add,
                replica_groups=[list(range(N_CORES))],
                ins=[cc_in.opt()], outs=[cc_out.opt()])
            total_sb = small.tile([1, D], F32)
            nc.sync.dma_start(total_sb[:], cc_out[:])

            # shared_upd = l2norm(total/8192); new_shared = .9*shared + .1*su
            sq2 = small.tile([1, D], F32)
            nrm2t = small.tile([1, 1], F32)
            nc.scalar.activation(sq2[:], total_sb[:],
                                 mybir.ActivationFunctionType.Square,
                                 accum_out=nrm2t[:])
            # mean_norm = sqrt(nrm2t) / B  ==  sqrt(nrm2t * (1/B^2))
            mnrm = small.tile([1, 1], F32)
            nc.scalar.activation(mnrm[:], nrm2t[:],
                                 mybir.ActivationFunctionType.Sqrt,
                                 scale=1.0 / (float(B) * float(B)))
            nc.vector.tensor_scalar_max(mnrm[:], mnrm[:], EPS_NORM)
            invt = small.tile([1, 1], F32)
            nc.vector.reciprocal(invt[:], mnrm[:])
            # su01 = total * (invt * 0.1 / B)   (= 0.1 * l2norm(mean))
            nc.vector.tensor_scalar_mul(invt[:], invt[:], (1.0 - MOM) / float(B))
            su01 = small.tile([1, D], F32)
            nc.vector.tensor_scalar_mul(su01[:], total_sb[:], invt[:])
            su16 = psump.tile([P, D], F32)
            nc.tensor.matmul(su16[:], lhsT=ones16[:], rhs=su01[:],
                             start=True, stop=True)
            nsh_t = small.tile([P, D], F32)
            nc.vector.scalar_tensor_tensor(
                nsh_t[:], shared_sb[:], MOM, su16[:],
                mybir.AluOpType.mult, mybir.AluOpType.add)
            nc.sync.dma_start(nsh_d[:, :], nsh_t[:])
        else:
            nc.sync.dma_start(partial_d[:, :], part_sb[:])
            # still write new_shared (host will overwrite); keep NEFF outputs
            nc.sync.dma_start(nsh_d[:, :], shared_sb[:])

        # ---- pass-through of untouched rows ----
        if pass_through:
            nc.sync.dma_start(pout_d[:, :], pin_d[:, :])
            nc.sync.dma_start(picout_d[:, :], pic_d[:, :])

    nc.compile()
    return nc


_CACHE: dict = {}

PASS_THROUGH = True
USE_COLLECTIVE = True
MUL_ENGINE = "vector"


def _get_module():
    key = (PASS_THROUGH, USE_COLLECTIVE, MUL_ENGINE)
    if key not in _CACHE:
        _CACHE[key] = build_module(PASS_THROUGH, USE_COLLECTIVE, MUL_ENGINE)
    return _CACHE[key]


def make_in_maps(user_prototypes, shared_prototypes, interaction_count,
                 features, success_mask, user_idx, pass_through=PASS_THROUGH):
    """Host-side routing: build the per-core input maps + assembly metadata."""
    up = np.ascontiguousarray(np.asarray(user_prototypes, dtype=np.float32))
    sp = np.ascontiguousarray(
        np.asarray(shared_prototypes, dtype=np.float32)).reshape(P, D)
    ic = np.ascontiguousarray(np.asarray(interaction_count, dtype=np.float32))
    ft = np.ascontiguousarray(np.asarray(features, dtype=np.float32))
    sm = np.ascontiguousarray(np.asarray(success_mask, dtype=np.float32))
    uidx = np.asarray(user_idx).astype(np.int64).ravel()

    touched = np.zeros(NUM_USERS, dtype=bool)
    touched[uidx] = True
    untouched = np.nonzero(~touched)[0]
    n_dev = min(len(untouched), PASS_TOTAL)
    dev_rows = untouched[:PASS_TOTAL]
    host_rows = untouched[PASS_TOTAL:]
    pad = PASS_TOTAL - n_dev  # only if user_idx has duplicates
    if pad:
        dev_rows = np.concatenate([dev_rows, np.zeros(pad, np.int64)])

    up2 = up.reshape(NUM_USERS, P * D)
    ft2 = ft.reshape(B, S * D)

    in_maps = []
    for d in range(N_CORES):
        bsl = slice(d * BPC, (d + 1) * BPC)
        rows = uidx[bsl]
        m = {
            "features": ft2[bsl],
            "mask": sm[bsl],
            "cur": up2[rows],
            "ic": ic[rows][:, None],
            "shared": sp,
        }
        if pass_through:
            prows = dev_rows[d * PASS_ROWS:(d + 1) * PASS_ROWS]
            m["pass_in"] = up2[prows]
            m["pass_ic_in"] = ic[prows][:, None]
        in_maps.append(m)
    meta = dict(uidx=uidx, dev_rows=dev_rows, host_rows=host_rows, pad=pad,
                up=up, ic=ic, sp=sp)
    return in_maps, meta


def assemble(results, meta, pass_through=PASS_THROUGH,
             use_collective=USE_COLLECTIVE):
    uidx, dev_rows, host_rows = meta["uidx"], meta["dev_rows"], meta["host_rows"]
    up, ic, sp = meta["up"], meta["ic"], meta["sp"]

    new_up = np.empty_like(up)                       # [NUM_USERS, P, D]
    new_ic = np.empty_like(ic)
    nu2 = new_up.reshape(NUM_USERS, P * D)
    if pass_through:
        for d in range(N_CORES):
            prows = dev_rows[d * PASS_ROWS:(d + 1) * PASS_ROWS]
            n = PASS_ROWS if not meta["pad"] else None
            nu2[prows] = results[d]["pass_out"]
            new_ic[prows] = np.asarray(results[d]["pass_ic_out"]).ravel()
        if meta["pad"]:
            # padded rows wrote garbage into row0 slot ordering; fix all
            nu2[dev_rows] = up.reshape(NUM_USERS, P * D)[dev_rows]
            new_ic[dev_rows] = ic[dev_rows]
    else:
        rows = np.concatenate([dev_rows, host_rows]) if len(host_rows) else dev_rows
        nu2[rows] = up.reshape(NUM_USERS, P * D)[rows]
        new_ic[rows] = ic[rows]
    if len(host_rows):
        nu2[host_rows] = up.reshape(NUM_USERS, P * D)[host_rows]
        new_ic[host_rows] = ic[host_rows]
    for d in range(N_CORES):
        bsl = slice(d * BPC, (d + 1) * BPC)
        rows = uidx[bsl]
        nu2[rows] = results[d]["new_batch"]
        new_ic[rows] = np.asarray(results[d]["new_ic"]).ravel()

    if use_collective:
        new_sh = results[0]["new_shared"].reshape(1, P, D).copy()
    else:
        total = np.sum([r["partial"][0] for r in results], axis=0)  # [D]
        mean = total / float(B)
        n = max(float(np.linalg.norm(mean)), EPS_NORM)
        su = mean / n
        new_sh = (MOM * sp + (1.0 - MOM) * su[None, :]).reshape(1, P, D)
        new_sh = new_sh.astype(np.float32)
    return new_up, new_sh, new_ic


def kernel(user_prototypes, shared_prototypes, interaction_count, features,
           success_mask, user_idx):
    nc = _get_module()
    in_maps, meta = make_in_maps(user_prototypes, shared_prototypes,
                                 interaction_count, features, success_mask,
                                 user_idx)
    res = bass_utils.run_bass_kernel_spmd(
        nc, in_maps, core_ids=list(range(N_CORES)))
    return assemble(res.results, meta)


# revision 31
# speedup vs baseline: 723.5845x; 723.5845x over previous
"""Trainium2 Bass kernel for nn_EnhancedMemoryManager (scatter_memory).

Contract: kernel(**inputs) takes FULL unsharded inputs and returns the FULL
outputs (new_user_prototypes, new_shared, new_ic), matching reference().

Sharding strategy (hardcoded for shapes:
  user_prototypes [50000,16,128], features [8192,50,128], success_mask
  [8192,50], interaction_count [50000], shared_prototypes [1,16,128],
  user_idx [8192]):
 - batch rows are split contiguously: core d gets rows [d*1024,(d+1)*1024);
   the prototype/ic rows for those users are routed (host-side gather) to
   that core (expert/data-parallel routing per user ownership).
 - the 41808 untouched user rows are split 5226/core and passed through the
   device (DMA copy) so the full output streams out of the 8 cores.
 - the shared_prototypes update (mean over the full batch) is AllReduced
   across the 8 cores; every core computes the final new_shared.
"""

from contextlib import ExitStack

import numpy as np

import concourse.bacc as bacc
import concourse.bass as bass
import concourse.mybir as mybir
import concourse.tile as tile
from concourse import bass_utils

N_CORES = 8
NUM_USERS, P, D = 50000, 16, 128
B, S = 8192, 50
BPC = B // N_CORES              # 1024 batch rows per core
NCHUNK = BPC // 128             # 8 chunks of 128 rows
PASS_TOTAL = NUM_USERS - B      # 41808 untouched rows
PASS_ROWS = PASS_TOTAL // N_CORES  # 5226 per core
MOM = 0.9
EPS_MASK = 1e-6
EPS_NORM = 1e-12
F32 = mybir.dt.float32


def build_module(pass_through: bool = True, use_collective: bool = True,
                 mul_engine: str = "vector", gp_mul_chunks: int = 0,
                 gp_blend_chunks: int = 0, repeat: int = 1,
                 act_small: bool = False, split_dma: bool = False):
    nc = bacc.Bacc("TRN2", target_bir_lowering=False, debug=False,
                   num_devices=N_CORES)

    feat_d = nc.dram_tensor("features", [BPC, S * D], F32, kind="ExternalInput")
    mask_d = nc.dram_tensor("mask", [BPC, S], F32, kind="ExternalInput")
    cur_d = nc.dram_tensor("cur", [BPC, P * D], F32, kind="ExternalInput")
    ic_d = nc.dram_tensor("ic", [BPC, 1], F32, kind="ExternalInput")
    shared_d = nc.dram_tensor("shared", [P, D], F32, kind="ExternalInput")
    nb_d = nc.dram_tensor("new_batch", [BPC, P * D], F32, kind="ExternalOutput")
    nic_d = nc.dram_tensor("new_ic", [BPC, 1], F32, kind="ExternalOutput")
    nsh_d = nc.dram_tensor("new_shared", [P, D], F32, kind="ExternalOutput")
    if pass_through:
        pin_d = nc.dram_tensor("pass_in", [PASS_ROWS, P * D], F32,
                               kind="ExternalInput")
        pic_d = nc.dram_tensor("pass_ic_in", [PASS_ROWS, 1], F32,
                               kind="ExternalInput")
        pout_d = nc.dram_tensor("pass_out", [PASS_ROWS, P * D], F32,
                                kind="ExternalOutput")
        picout_d = nc.dram_tensor("pass_ic_out", [PASS_ROWS, 1], F32,
                                  kind="ExternalOutput")
    if not use_collective:
        partial_d = nc.dram_tensor("partial", [1, D], F32, kind="ExternalOutput")

    mul_eng = {"vector": nc.vector, "gpsimd": nc.gpsimd}[mul_engine]
    st_eng = nc.scalar if split_dma else nc.sync

    with tile.TileContext(nc) as tc, ExitStack() as ctx:
        const = ctx.enter_context(tc.tile_pool(name="const", bufs=1))
        featp = ctx.enter_context(tc.tile_pool(name="featp", bufs=3))
        curp = ctx.enter_context(tc.tile_pool(name="curp", bufs=3))
        outp = ctx.enter_context(tc.tile_pool(name="outp", bufs=3))
        small = ctx.enter_context(tc.tile_pool(name="small", bufs=3))
        psump = ctx.enter_context(tc.tile_pool(name="psump", bufs=1,
                                               space="PSUM"))
        dramp = ctx.enter_context(tc.tile_pool(name="dramp", bufs=1,
                                               space="DRAM"))

        def emit():
            # whole-shard loads of the small tensors
            mask_sb = const.tile([128, NCHUNK * S], F32, name="mask_sb")
            nc.sync.dma_start(
                mask_sb[:].rearrange("p (k s) -> p k s", k=NCHUNK),
                mask_d[:, :].rearrange("(p k) s -> p k s", k=NCHUNK),
            )
            ic_sb = const.tile([128, NCHUNK], F32, name="ic_sb")
            nc.sync.dma_start(
                ic_sb[:], ic_d[:, :].rearrange("(p k) o -> p (k o)", k=NCHUNK))
            shared_sb = const.tile([P, D], F32, name="shared_sb")
            nc.sync.dma_start(shared_sb[:], shared_d[:, :])
            ones_sb = const.tile([128, 1], F32, name="ones_sb")
            nc.vector.memset(ones_sb[:], 1.0)
            ones16 = const.tile([1, P], F32, name="ones16")
            nc.vector.memset(ones16[:], 1.0)

            psum_part = psump.tile([1, D], F32, name="psum_part")

            # batched per-row scalars for ALL chunks at once: [128, NCHUNK]
            m_all = const.tile([128, NCHUNK], F32, name="m_all")
            nc.vector.tensor_scalar(m_all[:], ic_sb[:], 0.001, MOM,
                                    mybir.AluOpType.mult, mybir.AluOpType.add)
            nc.vector.tensor_scalar(m_all[:], m_all[:], MOM, 0.99,
                                    mybir.AluOpType.max, mybir.AluOpType.min)
            om_all = const.tile([128, NCHUNK], F32, name="om_all")
            nc.vector.tensor_scalar(om_all[:], m_all[:], -1.0, 1.0,
                                    mybir.AluOpType.mult, mybir.AluOpType.add)
            nic_all = const.tile([128, NCHUNK], F32, name="nic_all")
            nc.vector.tensor_scalar_add(nic_all[:], ic_sb[:], 1.0)
            nc.sync.dma_start(
                nic_d[:, :].rearrange("(p k) o -> p (k o)", k=NCHUNK), nic_all[:])
            # msum/inv for all chunks: [128, NCHUNK]
            invm_all = const.tile([128, NCHUNK], F32, name="invm_all")
            nc.vector.tensor_reduce(
                invm_all[:], mask_sb[:].rearrange("p (k s) -> p k s", k=NCHUNK),
                axis=mybir.AxisListType.X, op=mybir.AluOpType.add)
            nc.vector.tensor_scalar_max(invm_all[:], invm_all[:], EPS_MASK)
            nc.vector.reciprocal(invm_all[:], invm_all[:])

            for k in range(NCHUNK):
                feat_t = featp.tile([128, S * D], F32, name="feat_t")
                nc.sync.dma_start(feat_t[:], feat_d[:, :].rearrange("(p k) f -> k p f", k=NCHUNK)[k])
                cur_t = curp.tile([128, P * D], F32, name="cur_t")
                nc.sync.dma_start(cur_t[:], cur_d[:, :].rearrange("(p k) f -> k p f", k=NCHUNK)[k])

                mask_k = mask_sb[:, bass.ts(k, S)]                  # [128, S]
                m_t = m_all[:, k:k + 1]
                om_t = om_all[:, k:k + 1]

                # masked sum over history: prod = feat * mask (bcast over d)
                feat3 = feat_t[:].rearrange("p (s d) -> p s d", s=S)
                mask_b = mask_k.unsqueeze(2).broadcast_to([128, S, D])
                if gp_mul_chunks == -1:  # split each chunk's mul DVE/gpsimd
                    h = S // 2
                    nc.vector.tensor_tensor(
                        feat3[:, :h], feat3[:, :h], mask_b[:, :h],
                        mybir.AluOpType.mult)
                    nc.gpsimd.tensor_tensor(
                        feat3[:, h:], feat3[:, h:], mask_b[:, h:],
                        mybir.AluOpType.mult)
                else:
                    meng = nc.gpsimd if k < gp_mul_chunks else mul_eng
                    meng.tensor_tensor(feat3, feat3, mask_b,
                                       mybir.AluOpType.mult)

                # usum[p, d] = sum_s prod[p, s, d]
                usum = small.tile([128, D], F32, name="usum")
                nc.vector.tensor_reduce(
                    usum[:], feat_t[:].rearrange("p (s d) -> p d s", s=S),
                    axis=mybir.AxisListType.X, op=mybir.AluOpType.add)

                # upd = usum / max(msum, eps)
                upd = small.tile([128, D], F32, name="upd")
                if act_small:
                    nc.scalar.mul(upd[:], usum[:], invm_all[:, k:k + 1])
                else:
                    nc.vector.tensor_scalar_mul(upd[:], usum[:],
                                                invm_all[:, k:k + 1])

                # l2 normalize upd along d
                sq = small.tile([128, D], F32, name="sq")
                nrm2 = small.tile([128, 1], F32, name="nrm2")
                nc.scalar.activation(sq[:], upd[:],
                                     mybir.ActivationFunctionType.Square,
                                     accum_out=nrm2[:])
                nrm = small.tile([128, 1], F32, name="nrm")
                nc.scalar.sqrt(nrm[:], nrm2[:])
                nc.vector.tensor_scalar_max(nrm[:], nrm[:], EPS_NORM)
                invn = small.tile([128, 1], F32, name="invn")
                nc.vector.reciprocal(invn[:], nrm[:])
                updn = small.tile([128, D], F32, name="updn")
                if act_small:
                    nc.scalar.mul(updn[:], upd[:], invn[:])
                else:
                    nc.vector.tensor_scalar_mul(updn[:], upd[:], invn[:])

                # partial sum over batch rows of updn (for shared update): PE
                nc.tensor.matmul(psum_part[:], lhsT=ones_sb[:], rhs=updn[:],
                                 start=(k == 0), stop=(k == NCHUNK - 1))

                # v = updn * (1 - m); new = cur * m + v
                v_t = small.tile([128, D], F32, name="v_t")
                if act_small:
                    nc.scalar.mul(v_t[:], updn[:], om_t)
                else:
                    nc.vector.tensor_scalar_mul(v_t[:], updn[:], om_t)
                out_t = outp.tile([128, P * D], F32, name="out_t")
                beng = nc.gpsimd if k < gp_blend_chunks else nc.vector
                beng.scalar_tensor_tensor(
                    out_t[:].rearrange("p (q d) -> p q d", q=P),
                    cur_t[:].rearrange("p (q d) -> p q d", q=P),
                    m_t,
                    v_t[:].unsqueeze(1).broadcast_to([128, P, D]),
                    mybir.AluOpType.mult, mybir.AluOpType.add)
                st_eng.dma_start(nb_d[:, :].rearrange("(p k) f -> k p f", k=NCHUNK)[k], out_t[:])

            # ---- shared prototype update ----
            part_sb = small.tile([1, D], F32, name="part_sb")
            nc.vector.tensor_copy(part_sb[:], psum_part[:])

            if use_collective:
                cc_in = dramp.tile([1, D], F32, name="cc_in")
                cc_out = dramp.tile([1, D], F32, addr_space="Shared",
                                    name="cc_out")
                nc.sync.dma_start(cc_in[:], part_sb[:])
                nc.gpsimd.collective_compute(
                    "AllReduce", mybir.AluOpType.add,
                    replica_groups=[list(range(N_CORES))],
                    ins=[cc_in.opt()], outs=[cc_out.opt()])
                total_sb = small.tile([1, D], F32, name="total_sb")
                nc.sync.dma_start(total_sb[:], cc_out[:])

                # shared_upd = l2norm(total/B); new_shared = .9*sh + .1*su
                sq2 = small.tile([1, D], F32, name="sq2")
                nrm2t = small.tile([1, 1], F32, name="nrm2t")
                nc.scalar.activation(sq2[:], total_sb[:],
                                     mybir.ActivationFunctionType.Square,
                                     accum_out=nrm2t[:])
                # mean_norm = sqrt(nrm2t) / B  ==  sqrt(nrm2t * (1/B^2))
                mnrm = small.tile([1, 1], F32, name="mnrm")
                nc.scalar.activation(mnrm[:], nrm2t[:],
                                     mybir.ActivationFunctionType.Sqrt,
                                     scale=1.0 / (float(B) * float(B)))
                nc.vector.tensor_scalar_max(mnrm[:], mnrm[:], EPS_NORM)
                invt = small.tile([1, 1], F32, name="invt")
                nc.vector.reciprocal(invt[:], mnrm[:])
                # su01 = total * (invt * 0.1 / B)   (= 0.1 * l2norm(mean))
                nc.vector.tensor_scalar_mul(invt[:], invt[:],
                                            (1.0 - MOM) / float(B))
                su01 = small.tile([1, D], F32, name="su01")
                nc.vector.tensor_scalar_mul(su01[:], total_sb[:], invt[:])
                su16 = psump.tile([P, D], F32, name="su16")
                nc.tensor.matmul(su16[:], lhsT=ones16[:], rhs=su01[:],
                                 start=True, stop=True)
                nsh_t = small.tile([P, D], F32, name="nsh_t")
                nc.vector.scalar_tensor_tensor(
                    nsh_t[:], shared_sb[:], MOM, su16[:],
                    mybir.AluOpType.mult, mybir.AluOpType.add)
                nc.sync.dma_start(nsh_d[:, :], nsh_t[:])
            else:
                nc.sync.dma_start(partial_d[:, :], part_sb[:])
                # still write new_shared (host overwrites); keep NEFF outputs
                nc.sync.dma_start(nsh_d[:, :], shared_sb[:])

            # ---- pass-through of untouched rows ----
            if pass_through:
                nc.sync.dma_start(pout_d[:, :], pin_d[:, :])
                nc.sync.dma_start(picout_d[:, :], pic_d[:, :])

        for _ in range(repeat):
            emit()

    nc.compile()
    return nc


_CACHE: dict = {}

PASS_THROUGH = True
USE_COLLECTIVE = True
MUL_ENGINE = "vector"
GP_MUL_CHUNKS = 0
GP_BLEND_CHUNKS = 0


def _get_module(repeat: int = 1):
    key = (PASS_THROUGH, USE_COLLECTIVE, MUL_ENGINE, GP_MUL_CHUNKS,
           GP_BLEND_CHUNKS, repeat)
    if key not in _CACHE:
        _CACHE[key] = build_module(PASS_THROUGH, USE_COLLECTIVE, MUL_ENGINE,
                                   GP_MUL_CHUNKS, GP_BLEND_CHUNKS, repeat)
    return _CACHE[key]


def make_in_maps(user_prototypes, shared_prototypes, interaction_count,
                 features, success_mask, user_idx, pass_through=None):
    """Host-side routing: build the per-core input maps + assembly metadata."""
    if pass_through is None:
        pass_through = PASS_THROUGH
    up = np.ascontiguousarray(np.asarray(user_prototypes, dtype=np.float32))
    sp = np.ascontiguousarray(
        np.asarray(shared_prototypes, dtype=np.float32)).reshape(P, D)
    ic = np.ascontiguousarray(np.asarray(interaction_count, dtype=np.float32))
    ft = np.ascontiguousarray(np.asarray(features, dtype=np.float32))
    sm = np.ascontiguousarray(np.asarray(success_mask, dtype=np.float32))
    uidx = np.asarray(user_idx).astype(np.int64).ravel()

    touched = np.zeros(NUM_USERS, dtype=bool)
    touched[uidx] = True
    untouched = np.nonzero(~touched)[0]
    n_dev = min(len(untouched), PASS_TOTAL)
    dev_rows = untouched[:PASS_TOTAL]
    host_rows = untouched[PASS_TOTAL:]
    pad = PASS_TOTAL - n_dev  # only if user_idx has duplicates
    if pad:
        dev_rows = np.concatenate([dev_rows, np.zeros(pad, np.int64)])

    up2 = up.reshape(NUM_USERS, P * D)
    ft2 = ft.reshape(B, S * D)

    in_maps = []
    for d in range(N_CORES):
        bsl = slice(d * BPC, (d + 1) * BPC)
        rows = uidx[bsl]
        m = {
            "features": ft2[bsl],
            "mask": sm[bsl],
            "cur": up2[rows],
            "ic": ic[rows][:, None],
            "shared": sp,
        }
        if pass_through:
            prows = dev_rows[d * PASS_ROWS:(d + 1) * PASS_ROWS]
            m["pass_in"] = up2[prows]
            m["pass_ic_in"] = ic[prows][:, None]
        in_maps.append(m)
    meta = dict(uidx=uidx, dev_rows=dev_rows, host_rows=host_rows, pad=pad,
                up=up, ic=ic, sp=sp)
    return in_maps, meta


def assemble(results, meta, pass_through=None, use_collective=None):
    if pass_through is None:
        pass_through = PASS_THROUGH
    if use_collective is None:
        use_collective = USE_COLLECTIVE
    uidx, dev_rows, host_rows = meta["uidx"], meta["dev_rows"], meta["host_rows"]
    up, ic, sp = meta["up"], meta["ic"], meta["sp"]

    new_up = np.empty_like(up)                       # [NUM_USERS, P, D]
    new_ic = np.empty_like(ic)
    nu2 = new_up.reshape(NUM_USERS, P * D)
    if pass_through:
        for d in range(N_CORES):
            prows = dev_rows[d * PASS_ROWS:(d + 1) * PASS_ROWS]
            nu2[prows] = results[d]["pass_out"]
            new_ic[prows] = np.asarray(results[d]["pass_ic_out"]).ravel()
        if meta["pad"]:
            # padded slots wrote duplicate rows; restore untouched from input
            nu2[dev_rows] = up.reshape(NUM_USERS, P * D)[dev_rows]
            new_ic[dev_rows] = ic[dev_rows]
    else:
        nu2[dev_rows] = up.reshape(NUM_USERS, P * D)[dev_rows]
        new_ic[dev_rows] = ic[dev_rows]
    if len(host_rows):
        nu2[host_rows] = up.reshape(NUM_USERS, P * D)[host_rows]
        new_ic[host_rows] = ic[host_rows]
    for d in range(N_CORES):
        bsl = slice(d * BPC, (d + 1) * BPC)
        rows = uidx[bsl]
        nu2[rows] = results[d]["new_batch"]
        new_ic[rows] = np.asarray(results[d]["new_ic"]).ravel()

    if use_collective:
        new_sh = np.asarray(results[0]["new_shared"],
                            dtype=np.float32).reshape(1, P, D).copy()
    else:
        total = np.sum([np.asarray(r["partial"]).reshape(D) for r in results],
                       axis=0)
        mean = total / float(B)
        n = max(float(np.linalg.norm(mean)), EPS_NORM)
        su = mean / n
        new_sh = (MOM * sp + (1.0 - MOM) * su[None, :]).reshape(1, P, D)
        new_sh = new_sh.astype(np.float32)
    return new_up, new_sh, new_ic


def kernel(user_prototypes, shared_prototypes, interaction_count, features,
           success_mask, user_idx):
    nc = _get_module()
    in_maps, meta = make_in_maps(user_prototypes, shared_prototypes,
                                 interaction_count, features, success_mask,
                                 user_idx)
    res = bass_utils.run_bass_kernel_spmd(
        nc, in_maps, core_ids=list(range(N_CORES)))
    return assemble(res.results, meta)
